# revision 1
# baseline (speedup 1.0000x reference)
"""Binary conv forward kernel for Trainium2 (8 NeuronCores, data-parallel over batch).

Computes y = conv2d(sign(x), scale[o] * sign(w)), stride 1, pad 1, NCHW/OIHW,
x [16, 64, 224, 224] f32, w [64*64*3*3, 1] f32 -> y [16, 64, 224, 224] f32.

Sharding: batch 16 -> 2 images per core, weights replicated (tiny).

Host side packs x into a tile-major layout [pair, 128, batch, w] (pure
relayout) so that every device DMA is a dense 128-partition 3-dim copy running
at full 16-SDMA-engine rate; the output is produced in a matching packed
layout and unpacked after the gather.

Device algorithm (per core, n_batch=2 images):
  - A resident fp8 "sign plane" holds sign(x) for the whole shard: slot j =
    rows (2j, 2j+1) (even row on partitions 0..63, odd on 64..127), both batch
    images in the free dim, one zero pad column each side for the kw shifts.
    sign() runs on ScalarE; -1/0/+1 are exact in fp8e4.
  - Interior output pair (2m+1, 2m+2) accumulates in PSUM [128, 2, 224] via 3
    DoubleRow matmuls (virtual K=256 over slots m, m+1; M=128; N=448), one per
    kw shift.  The stationary operand stacks the two block matrices
    [[W0,0],[W1,W0]] (slot m) and [[W2,W1],[0,W2]] (slot m+1), where
    Wk = sign(w)[:,:,kh=k,kw]^T.  Boundary rows 0 and 223 form one extra unit
    over slots 111 (V_111) and 112 (a copy of V_0) with blocks
    [[0,W0],[0,W1]] and [[W1,0],[W2,0]].
  - Two units share one 2-bank PSUM tile; VectorE evicts both at once with a
    per-partition scale[o] multiplier (scale = mean|w| per output channel,
    computed on device).
  - Input DMAs ride the HWDGE ring (nc.sync); weight + output DMAs ride SWDGE
    (nc.gpsimd) so loads and stores overlap on HBM.
"""

import numpy as np

import concourse.bacc as bacc
import concourse.mybir as mybir
import concourse.tile as tile

F32 = mybir.dt.float32
FP8 = mybir.dt.float8e4

N_CORES = 8
FULL_BATCH = 16
C = 64  # in channels == out channels
H = 224
W = 224
KH = KW = 3
# Sign-plane slot layout (fp8, per partition): [pad, b0 w=224, 0, b1 w=224,
# pad, pad] -> both batch images form one contiguous 450-wide matmul N strip;
# the shared zero column between them keeps the kw shifts exact.  452 cols
# used, padded to 464 (multiple of 16 for DoubleRow AP steps).
SW = 464   # slot stride
SN = 450   # matmul N (448 real output columns + 2 junk)
B0 = 1     # b0 image at cols 1..224
B1 = 226   # b1 image at cols 226..449


def build_nc(n_batch=2, h=H, w=W, g=16, enable_asserts=False, ic_bufs=3):
    """Build the single-core Bass module (same NEFF runs on all 8 cores)."""
    nc = bacc.Bacc(
        "TRN2",
        target_bir_lowering=False,
        debug=False,
        enable_asserts=enable_asserts,
    )
    assert h % 2 == 0
    NV = h // 2          # input row-pair slots, also output units
    assert NV % 2 == 0, "units are evicted in pairs"

    # Packed tensors: [pair, 128, batch, w].
    xp = nc.dram_tensor("xp", [NV, 128, n_batch, w], F32, kind="ExternalInput")
    wraw = nc.dram_tensor("wraw", [C * C * KH * KW, 1], F32, kind="ExternalInput")
    # wblk: host-arranged raw f32 weights in the 6-tile DoubleRow block layout
    # [128, (3 interior + 3 boundary) tiles, 2, 128] with zeros in the zero
    # blocks (pure replication/padding; sign is computed on device and
    # sign(0) = 0 keeps the zero blocks zero).
    wblk = nc.dram_tensor("wblk", [128, 6 * 2 * 128], F32, kind="ExternalInput")
    yp = nc.dram_tensor("yp", [NV, 128, n_batch, w], F32, kind="ExternalOutput")

    xr = xp.ap().rearrange("j p b w -> p j (b w)")   # [128, NV, n_batch*w]
    yr = yp.ap().rearrange("j p b w -> p j (b w)")

    with tile.TileContext(nc) as tc:
        with (
            tc.tile_pool(name="wpool", bufs=1) as wpool,
            tc.tile_pool(name="icpool", bufs=ic_bufs) as icpool,
            tc.tile_pool(name="pspool", bufs=3, space="PSUM") as pspool,
            tc.tile_pool(name="ocpool", bufs=2) as ocpool,
        ):
            # Input chunk schedule: small leading chunks so signing (and the
            # first matmuls) start as early as possible, then full chunks.
            ramp = [4, 8]
            starts = [0]
            ci_sizes = []
            while starts[-1] < NV:
                size = ramp[len(starts) - 1] if len(starts) <= len(ramp) else g
                size = min(size, NV - starts[-1])
                ci_sizes.append(size)
                starts.append(starts[-1] + size)
            starts.pop()
            chunk_of = {}
            for ci, s in enumerate(starts):
                for j in range(s, s + ci_sizes[ci]):
                    chunk_of[j] = (ci, s)

            # ---- weight prep (one-time, tiny) ----
            # One DMA + one Sign activation produces all six DoubleRow
            # stationary operands at once (zero blocks stay zero).  This DMA
            # rides the fast HWDGE ring ahead of the first input chunk: it
            # gates the very first matmul.
            wblkf = wpool.tile([128, 6, 2, 128], F32)
            nc.sync.dma_start(
                wblkf[:], wblk.ap().rearrange("p (t i m) -> p t i m", t=6, i=2)
            )
            sblk = wpool.tile([128, 6, 2, 128], FP8)
            nc.scalar.sign(sblk[:], wblkf[:])
            wdr = [sblk[:, kw, :, :] for kw in range(3)]
            wb = [sblk[:, 3 + kw, :, :] for kw in range(3)]

            # Prefetch the first input chunk.
            ic = icpool.tile([128, g, n_batch * w], F32, tag="ic", name="ic")
            nc.sync.dma_start(ic[:, 0 : ci_sizes[0], :], xr[:, 0 : ci_sizes[0], :])

            # scale[o] = mean(|w[o, :, :, :]|), O on partitions, duplicated on
            # both partition halves for the [128]-row eviction.  The ACT ops
            # for it are emitted inside the loop (at j == 1) so they don't sit
            # between the weight sign and the first row signs in ACT order.
            w2 = wpool.tile([128, 576], F32)
            wr = wraw.ap().rearrange("(o f) one -> o (f one)", o=C)
            nc.gpsimd.dma_start(w2[0:64], wr)
            nc.gpsimd.dma_start(w2[64:128], wr)
            absw = wpool.tile([128, 576], F32)
            sc_sum = wpool.tile([128, 1], F32)
            sc128 = wpool.tile([128, 1], F32)

            # Resident sign plane [128, NV+1, SW] fp8; slot NV = V_0 copy.
            # Zero pad columns once (plane slots are written exactly once):
            # col 0 (left pad), col 225 (separator / b0 right pad), cols
            # 450-451 (right pads, also read by the junk output column).
            assert n_batch == 2
            plane = wpool.tile([128, NV + 1, SW], FP8)
            nc.vector.memset(plane[:, :, 0:1], 0.0)
            nc.vector.memset(plane[:, :, 225:226], 0.0)
            nc.vector.memset(plane[:, :, 450:452], 0.0)

            def cp(dst, src):
                nc.vector.tensor_copy(out=dst, in_=src)

            def rhs(j, kw):
                return plane[:, j : j + 2, kw : kw + SN]

            def evict(ps, oc, jj):
                # psum cols 0..223 = b0, 225..448 = b1 (stride-225 blocks)
                nc.vector.tensor_scalar_mul(
                    oc[:, jj : jj + 2, :].rearrange(
                        "p j (b w) -> p j b w", b=n_batch
                    ),
                    ps[:, :, 0:450].rearrange("p u (b w) -> p u b w", w=225)[
                        :, :, :, 0:w
                    ],
                    sc128[:],
                )

            og = min(8, NV)  # output chunk size (earlier, shorter stores)
            assert og % 2 == 0
            oc = None
            oc_m0 = 0
            ps = None

            for j in range(NV):
                ci, cstart = chunk_of[j]
                if j == cstart and j > 0:
                    gc = ci_sizes[ci]
                    ic = icpool.tile([128, g, n_batch * w], F32, tag="ic", name="ic")
                    nc.sync.dma_start(ic[:, 0:gc, :], xr[:, j : j + gc, :])

                nc.scalar.sign(
                    plane[:, j, 1:451].rearrange("p (b w) -> p b w", w=225)[
                        :, :, 0:w
                    ],
                    ic[:, j - cstart, :].rearrange("p (b w) -> p b w", b=n_batch),
                )
                if j == 0:
                    cp(plane[:, NV, 0:452], plane[:, 0, 0:452])
                if j == 1:
                    nc.scalar.activation(
                        out=absw[:], in_=w2[:],
                        func=mybir.ActivationFunctionType.Abs,
                        accum_out=sc_sum[:],
                    )
                    nc.scalar.mul(sc128[:], sc_sum[:], 1.0 / 576.0)

                if j >= 1:
                    m = j - 1  # interior unit -> output rows (2m+1, 2m+2)
                    if m % og == 0:
                        oc = ocpool.tile(
                            [128, og, n_batch * w], F32, tag="oc", name="oc"
                        )
                        oc_m0 = m
                    if m % 2 == 0:
                        # per-unit stride padded to one full PSUM bank (2 KB)
                        ps = pspool.tile([128, 2, 512], F32, tag="ps", name="ps")
                    for kw in range(3):
                        nc.tensor.matmul(
                            ps[:, m % 2, 0:SN], wdr[kw][:], rhs(m, kw),
                            start=(kw == 0), stop=(kw == 2),
                            perf_mode=mybir.MatmulPerfMode.DoubleRow,
                        )
                    if m % 2 == 1:
                        evict(ps, oc, m - 1 - oc_m0)
                    if m == oc_m0 + og - 1:
                        nc.gpsimd.dma_start(yr[:, oc_m0 : m + 1, :], oc[:, 0:og, :])

            # Boundary unit (unit NV-1): rows 0 and h-1 via slots NV-1 and NV.
            m = NV - 1
            if m % og == 0:
                oc = ocpool.tile([128, og, n_batch * w], F32, tag="oc", name="oc")
                oc_m0 = m
            if m % 2 == 0:
                ps = pspool.tile([128, 2, 512], F32, tag="ps", name="ps")
            for kw in range(3):
                nc.tensor.matmul(
                    ps[:, m % 2, 0:SN], wb[kw][:], rhs(NV - 1, kw),
                    start=(kw == 0), stop=(kw == 2),
                    perf_mode=mybir.MatmulPerfMode.DoubleRow,
                )
            if m % 2 == 1:
                evict(ps, oc, m - 1 - oc_m0)
            else:
                # odd NV: evict the single last unit
                nc.vector.tensor_scalar_mul(
                    oc[:, m - oc_m0 : m - oc_m0 + 1, :].rearrange(
                        "p j (b w) -> p j b w", b=n_batch
                    ),
                    ps[:, m % 2 : m % 2 + 1, 0:450].rearrange(
                        "p u (b w) -> p u b w", w=225
                    )[:, :, :, 0:w],
                    sc128[:],
                )
            nc.gpsimd.dma_start(yr[:, oc_m0 : m + 1, :], oc[:, 0 : m - oc_m0 + 1, :])

    nc.compile()
    return nc


_NC_CACHE = {}


def _get_nc(key=(2, H, W, 16)):
    if key not in _NC_CACHE:
        _NC_CACHE[key] = build_nc(*key)
    return _NC_CACHE[key]


def _make_wblk(weights):
    """Arrange raw f32 weights into the 6-tile DoubleRow block layout
    [128, 6, 2, 128] (pure replication/zero-padding; sign runs on device)."""
    wt = weights.reshape(C, C, KH, KW).transpose(1, 2, 3, 0)  # [i, kh, kw, o]

    def T(kh, kw):
        return wt[:, kh, kw, :]  # W_{kh,kw}^T as [i, o]

    blk = np.zeros((128, 6, 2, 128), np.float32)
    for kw in range(KW):
        # interior tiles: i=0 -> [[W0, 0], [W1, W0]], i=1 -> [[W2, W1], [0, W2]]
        blk[0:64, kw, 0, 0:64] = T(0, kw)
        blk[64:128, kw, 0, 0:64] = T(1, kw)
        blk[64:128, kw, 0, 64:128] = T(0, kw)
        blk[0:64, kw, 1, 0:64] = T(2, kw)
        blk[0:64, kw, 1, 64:128] = T(1, kw)
        blk[64:128, kw, 1, 64:128] = T(2, kw)
        # boundary tiles: i=0 -> [[0, W0], [0, W1]], i=1 -> [[W1, 0], [W2, 0]]
        blk[0:64, 3 + kw, 0, 64:128] = T(0, kw)
        blk[64:128, 3 + kw, 0, 64:128] = T(1, kw)
        blk[0:64, 3 + kw, 1, 0:64] = T(1, kw)
        blk[64:128, 3 + kw, 1, 0:64] = T(2, kw)
    return blk.reshape(128, 6 * 2 * 128)


def pack_x(x_shard, h=H, w=W):
    """[nb, C, h, w] -> [h/2, 128, nb, w]; p = parity*64 + channel."""
    nb = x_shard.shape[0]
    xs = x_shard.reshape(nb, C, h // 2, 2, w)
    return np.ascontiguousarray(xs.transpose(2, 3, 1, 0, 4)).reshape(
        h // 2, 128, nb, w
    )


def unpack_y(ypk, h=H, w=W):
    """[h/2, 128, nb, w] -> [nb, C, h, w] per the unit layout."""
    NV = h // 2
    nb = ypk.shape[2]
    y = np.empty((nb, C, h, w), np.float32)
    # interior units m=0..NV-2 -> rows 2m+1 (p<64) and 2m+2 (p>=64)
    interior = ypk[: NV - 1].reshape(NV - 1, 2, C, nb, w)
    y[:, :, 1 : h - 1, :] = interior.transpose(3, 2, 0, 1, 4).reshape(
        nb, C, h - 2, w
    )
    # boundary unit: p<64 -> row 0, p>=64 -> row h-1
    y[:, :, 0, :] = ypk[NV - 1, 0:C].transpose(1, 0, 2)
    y[:, :, h - 1, :] = ypk[NV - 1, C:128].transpose(1, 0, 2)
    return y


def make_in_maps(x, weights):
    x = np.asarray(x, dtype=np.float32)
    weights = np.asarray(weights, dtype=np.float32)
    wblk = _make_wblk(weights)
    nb = FULL_BATCH // N_CORES
    return [
        {
            "xp": pack_x(x[c * nb : (c + 1) * nb]),
            "wraw": weights,
            "wblk": wblk,
        }
        for c in range(N_CORES)
    ]


def gather_out(results):
    return np.concatenate([unpack_y(r["yp"]) for r in results], axis=0)


def kernel(x, weights):
    from concourse import bass_utils

    nc = _get_nc()
    in_maps = make_in_maps(x, weights)
    res = bass_utils.run_bass_kernel_spmd(nc, in_maps, core_ids=list(range(N_CORES)))
    return gather_out(res.results)



# revision 4
# speedup vs baseline: 2.3684x; 2.3684x over previous
"""Binary conv forward kernel for Trainium2 (8 NeuronCores, data-parallel over batch).

Computes y = conv2d(sign(x), scale[o] * sign(w)), stride 1, pad 1, NCHW/OIHW,
x [16, 64, 224, 224] f32, w [64*64*3*3, 1] f32 -> y [16, 64, 224, 224] f32.

Sharding: batch 16 -> 2 images per core, weights replicated (tiny).

Host side packs x into a tile-major layout [pair, 128, batch, w] (pure
relayout) so that every device DMA is a dense 128-partition 3-dim copy running
at full 16-SDMA-engine rate; the output is produced in a matching packed
layout and unpacked after the gather.

Device algorithm (per core, n_batch=2 images):
  - A resident fp8 "sign plane" holds sign(x) for the whole shard: slot j =
    rows (2j, 2j+1) (even row on partitions 0..63, odd on 64..127), both batch
    images in the free dim, one zero pad column each side for the kw shifts.
    sign() runs on ScalarE; -1/0/+1 are exact in fp8e4.
  - Interior output pair (2m+1, 2m+2) accumulates in PSUM [128, 2, 224] via 3
    DoubleRow matmuls (virtual K=256 over slots m, m+1; M=128; N=448), one per
    kw shift.  The stationary operand stacks the two block matrices
    [[W0,0],[W1,W0]] (slot m) and [[W2,W1],[0,W2]] (slot m+1), where
    Wk = sign(w)[:,:,kh=k,kw]^T.  Boundary rows 0 and 223 form one extra unit
    over slots 111 (V_111) and 112 (a copy of V_0) with blocks
    [[0,W0],[0,W1]] and [[W1,0],[W2,0]].
  - Two units share one 2-bank PSUM tile; VectorE evicts both at once with a
    per-partition scale[o] multiplier (scale = mean|w| per output channel,
    computed on device).
  - Input DMAs ride the HWDGE ring (nc.sync); weight + output DMAs ride SWDGE
    (nc.gpsimd) so loads and stores overlap on HBM.
"""

import numpy as np

import concourse.bacc as bacc
import concourse.mybir as mybir
import concourse.tile as tile

F32 = mybir.dt.float32
F16 = mybir.dt.float16
FP8 = mybir.dt.float8e4

N_CORES = 8
FULL_BATCH = 16
C = 64  # in channels == out channels
H = 224
W = 224
KH = KW = 3
# Sign-plane slot layout (fp8, per partition): [pad, b0 w=224, 0, b1 w=224,
# pad, pad] -> both batch images form one contiguous 450-wide matmul N strip;
# the shared zero column between them keeps the kw shifts exact.  452 cols
# used, padded to 464 (multiple of 16 for DoubleRow AP steps).
SW = 464   # slot stride
SN = 450   # matmul N (448 real output columns + 2 junk)
B0 = 1     # b0 image at cols 1..224
B1 = 226   # b1 image at cols 226..449


def build_nc(n_batch=2, h=H, w=W, g=16, enable_asserts=False, ic_bufs=3):
    """Build the single-core Bass module (same NEFF runs on all 8 cores)."""
    nc = bacc.Bacc(
        "TRN2",
        target_bir_lowering=False,
        debug=False,
        enable_asserts=enable_asserts,
    )
    assert h % 2 == 0
    NV = h // 2          # input row-pair slots, also output units
    assert NV % 2 == 0, "units are evicted in pairs"

    # Packed tensors: [pair, 128, batch, w].
    xp = nc.dram_tensor("xp", [NV, 128, n_batch, w], F32, kind="ExternalInput")
    wraw = nc.dram_tensor("wraw", [C * C * KH * KW, 1], F32, kind="ExternalInput")
    # wblk: host-arranged raw f32 weights in the 6-tile DoubleRow block layout
    # [128, (3 interior + 3 boundary) tiles, 2, 128] with zeros in the zero
    # blocks (pure replication/padding; sign is computed on device and
    # sign(0) = 0 keeps the zero blocks zero).
    wblk = nc.dram_tensor("wblk", [128, 6 * 2 * 128], F32, kind="ExternalInput")
    # Output rides HBM in fp16: y = k * scale[o] with integer k, |k| <= 576,
    # so fp16 rounding (~5e-4 rel) is far inside the 2e-2 gate and halves
    # output DMA traffic on the shared DMA bus.
    yp = nc.dram_tensor("yp", [NV, 128, n_batch, w], F16, kind="ExternalOutput")

    xr = xp.ap().rearrange("j p b w -> p j (b w)")   # [128, NV, n_batch*w]
    yr = yp.ap().rearrange("j p b w -> p j (b w)")

    with tile.TileContext(nc) as tc:
        with (
            tc.tile_pool(name="wpool", bufs=1) as wpool,
            tc.tile_pool(name="icpool", bufs=ic_bufs) as icpool,
            tc.tile_pool(name="pspool", bufs=3, space="PSUM") as pspool,
            tc.tile_pool(name="ocpool", bufs=2) as ocpool,
        ):
            # Input chunk schedule: small leading chunks so signing (and the
            # first matmuls) start as early as possible, then full chunks.
            ramp = [4, 8]
            starts = [0]
            ci_sizes = []
            while starts[-1] < NV:
                size = ramp[len(starts) - 1] if len(starts) <= len(ramp) else g
                size = min(size, NV - starts[-1])
                ci_sizes.append(size)
                starts.append(starts[-1] + size)
            starts.pop()
            chunk_of = {}
            for ci, s in enumerate(starts):
                for j in range(s, s + ci_sizes[ci]):
                    chunk_of[j] = (ci, s)

            # ---- weight prep (one-time, tiny) ----
            # One DMA + one Sign activation produces all six DoubleRow
            # stationary operands at once (zero blocks stay zero).  This DMA
            # rides the fast HWDGE ring ahead of the first input chunk: it
            # gates the very first matmul.
            wblkf = wpool.tile([128, 6, 2, 128], F32)
            nc.sync.dma_start(
                wblkf[:], wblk.ap().rearrange("p (t i m) -> p t i m", t=6, i=2)
            )
            sblk = wpool.tile([128, 6, 2, 128], FP8)
            nc.scalar.sign(sblk[:], wblkf[:])
            wdr = [sblk[:, kw, :, :] for kw in range(3)]
            wb = [sblk[:, 3 + kw, :, :] for kw in range(3)]

            # Prefetch the first input chunk.
            ic = icpool.tile([128, g, n_batch * w], F32, tag="ic", name="ic")
            nc.sync.dma_start(ic[:, 0 : ci_sizes[0], :], xr[:, 0 : ci_sizes[0], :])

            # scale[o] = mean(|w[o, :, :, :]|), O on partitions, duplicated on
            # both partition halves for the [128]-row eviction.  The ACT ops
            # for it are emitted inside the loop (at j == 1) so they don't sit
            # between the weight sign and the first row signs in ACT order.
            w2 = wpool.tile([128, 576], F32)
            wr = wraw.ap().rearrange("(o f) one -> o (f one)", o=C)
            nc.gpsimd.dma_start(w2[0:64], wr)
            nc.gpsimd.dma_start(w2[64:128], wr)
            absw = wpool.tile([128, 576], F32)
            sc_sum = wpool.tile([128, 1], F32)
            sc128 = wpool.tile([128, 1], F32)

            # Resident sign plane [128, NV+1, SW] fp8; slot NV = V_0 copy.
            # Zero pad columns once (plane slots are written exactly once):
            # col 0 (left pad), col 225 (separator / b0 right pad), cols
            # 450-451 (right pads, also read by the junk output column).
            assert n_batch == 2
            plane = wpool.tile([128, NV + 1, SW], FP8)
            nc.vector.memset(plane[:, :, 0:1], 0.0)
            nc.vector.memset(plane[:, :, 225:226], 0.0)
            nc.vector.memset(plane[:, :, 450:452], 0.0)

            def cp(dst, src):
                nc.vector.tensor_copy(out=dst, in_=src)

            def rhs(j, kw):
                return plane[:, j : j + 2, kw : kw + SN]

            def evict(ps, oc, jj):
                # psum cols 0..223 = b0, 225..448 = b1 (stride-225 blocks)
                nc.vector.tensor_scalar_mul(
                    oc[:, jj : jj + 2, :].rearrange(
                        "p j (b w) -> p j b w", b=n_batch
                    ),
                    ps[:, :, 0:450].rearrange("p u (b w) -> p u b w", w=225)[
                        :, :, :, 0:w
                    ],
                    sc128[:],
                )

            og = min(8, NV)  # output chunk size (earlier, shorter stores)
            assert og % 2 == 0
            oc = None
            oc_m0 = 0
            ps = None

            for j in range(NV):
                ci, cstart = chunk_of[j]
                if j == cstart and j > 0:
                    gc = ci_sizes[ci]
                    ic = icpool.tile([128, g, n_batch * w], F32, tag="ic", name="ic")
                    nc.sync.dma_start(ic[:, 0:gc, :], xr[:, j : j + gc, :])

                nc.scalar.sign(
                    plane[:, j, 1:451].rearrange("p (b w) -> p b w", w=225)[
                        :, :, 0:w
                    ],
                    ic[:, j - cstart, :].rearrange("p (b w) -> p b w", b=n_batch),
                )
                if j == 0:
                    cp(plane[:, NV, 0:452], plane[:, 0, 0:452])
                if j == 1:
                    nc.scalar.activation(
                        out=absw[:], in_=w2[:],
                        func=mybir.ActivationFunctionType.Abs,
                        accum_out=sc_sum[:],
                    )
                    nc.scalar.mul(sc128[:], sc_sum[:], 1.0 / 576.0)

                if j >= 1:
                    m = j - 1  # interior unit -> output rows (2m+1, 2m+2)
                    if m % og == 0:
                        oc = ocpool.tile(
                            [128, og, n_batch * w], F16, tag="oc", name="oc"
                        )
                        oc_m0 = m
                    if m % 2 == 0:
                        # per-unit stride padded to one full PSUM bank (2 KB)
                        ps = pspool.tile([128, 2, 512], F32, tag="ps", name="ps")
                    for kw in range(3):
                        nc.tensor.matmul(
                            ps[:, m % 2, 0:SN], wdr[kw][:], rhs(m, kw),
                            start=(kw == 0), stop=(kw == 2),
                            perf_mode=mybir.MatmulPerfMode.DoubleRow,
                        )
                    if m % 2 == 1:
                        evict(ps, oc, m - 1 - oc_m0)
                    if m == oc_m0 + og - 1:
                        nc.gpsimd.dma_start(yr[:, oc_m0 : m + 1, :], oc[:, 0:og, :])

            # Boundary unit (unit NV-1): rows 0 and h-1 via slots NV-1 and NV.
            m = NV - 1
            if m % og == 0:
                oc = ocpool.tile([128, og, n_batch * w], F16, tag="oc", name="oc")
                oc_m0 = m
            if m % 2 == 0:
                ps = pspool.tile([128, 2, 512], F32, tag="ps", name="ps")
            for kw in range(3):
                nc.tensor.matmul(
                    ps[:, m % 2, 0:SN], wb[kw][:], rhs(NV - 1, kw),
                    start=(kw == 0), stop=(kw == 2),
                    perf_mode=mybir.MatmulPerfMode.DoubleRow,
                )
            if m % 2 == 1:
                evict(ps, oc, m - 1 - oc_m0)
            else:
                # odd NV: evict the single last unit
                nc.vector.tensor_scalar_mul(
                    oc[:, m - oc_m0 : m - oc_m0 + 1, :].rearrange(
                        "p j (b w) -> p j b w", b=n_batch
                    ),
                    ps[:, m % 2 : m % 2 + 1, 0:450].rearrange(
                        "p u (b w) -> p u b w", w=225
                    )[:, :, :, 0:w],
                    sc128[:],
                )
            nc.gpsimd.dma_start(yr[:, oc_m0 : m + 1, :], oc[:, 0 : m - oc_m0 + 1, :])

    nc.compile()
    return nc


_NC_CACHE = {}


def _get_nc(key=(2, H, W, 16)):
    if key not in _NC_CACHE:
        _NC_CACHE[key] = build_nc(*key)
    return _NC_CACHE[key]


def _make_wblk(weights):
    """Arrange raw f32 weights into the 6-tile DoubleRow block layout
    [128, 6, 2, 128] (pure replication/zero-padding; sign runs on device)."""
    wt = weights.reshape(C, C, KH, KW).transpose(1, 2, 3, 0)  # [i, kh, kw, o]

    def T(kh, kw):
        return wt[:, kh, kw, :]  # W_{kh,kw}^T as [i, o]

    blk = np.zeros((128, 6, 2, 128), np.float32)
    for kw in range(KW):
        # interior tiles: i=0 -> [[W0, 0], [W1, W0]], i=1 -> [[W2, W1], [0, W2]]
        blk[0:64, kw, 0, 0:64] = T(0, kw)
        blk[64:128, kw, 0, 0:64] = T(1, kw)
        blk[64:128, kw, 0, 64:128] = T(0, kw)
        blk[0:64, kw, 1, 0:64] = T(2, kw)
        blk[0:64, kw, 1, 64:128] = T(1, kw)
        blk[64:128, kw, 1, 64:128] = T(2, kw)
        # boundary tiles: i=0 -> [[0, W0], [0, W1]], i=1 -> [[W1, 0], [W2, 0]]
        blk[0:64, 3 + kw, 0, 64:128] = T(0, kw)
        blk[64:128, 3 + kw, 0, 64:128] = T(1, kw)
        blk[0:64, 3 + kw, 1, 0:64] = T(1, kw)
        blk[64:128, 3 + kw, 1, 0:64] = T(2, kw)
    return blk.reshape(128, 6 * 2 * 128)


def pack_x(x_shard, h=H, w=W):
    """[nb, C, h, w] -> [h/2, 128, nb, w]; p = parity*64 + channel."""
    nb = x_shard.shape[0]
    xs = x_shard.reshape(nb, C, h // 2, 2, w)
    return np.ascontiguousarray(xs.transpose(2, 3, 1, 0, 4)).reshape(
        h // 2, 128, nb, w
    )


def unpack_y(ypk, h=H, w=W):
    """[h/2, 128, nb, w] -> [nb, C, h, w] per the unit layout."""
    NV = h // 2
    nb = ypk.shape[2]
    y = np.empty((nb, C, h, w), np.float32)
    # interior units m=0..NV-2 -> rows 2m+1 (p<64) and 2m+2 (p>=64)
    interior = ypk[: NV - 1].reshape(NV - 1, 2, C, nb, w)
    y[:, :, 1 : h - 1, :] = interior.transpose(3, 2, 0, 1, 4).reshape(
        nb, C, h - 2, w
    )
    # boundary unit: p<64 -> row 0, p>=64 -> row h-1
    y[:, :, 0, :] = ypk[NV - 1, 0:C].transpose(1, 0, 2)
    y[:, :, h - 1, :] = ypk[NV - 1, C:128].transpose(1, 0, 2)
    return y


def make_in_maps(x, weights):
    x = np.asarray(x, dtype=np.float32)
    weights = np.asarray(weights, dtype=np.float32)
    wblk = _make_wblk(weights)
    nb = FULL_BATCH // N_CORES
    return [
        {
            "xp": pack_x(x[c * nb : (c + 1) * nb]),
            "wraw": weights,
            "wblk": wblk,
        }
        for c in range(N_CORES)
    ]


def gather_out(results):
    return np.concatenate([unpack_y(r["yp"]) for r in results], axis=0)


def kernel(x, weights):
    from concourse import bass_utils

    nc = _get_nc()
    in_maps = make_in_maps(x, weights)
    res = bass_utils.run_bass_kernel_spmd(nc, in_maps, core_ids=list(range(N_CORES)))
    return gather_out(res.results)

